# revision 28
# baseline (speedup 1.0000x reference)
"""Trainium2 Bass kernel for nn_KVCacheMoE (B=8, S=2048, H=1024, E=8).

Batch-parallel across the 8 NeuronCores (core c owns batch c) - the router
depends only on that batch, so no collectives.

Experts 0,2,3,4,5,6 run in fp8-e4m3 with perf_mode=DoubleRow (2.0x PE
throughput measured vs bf16); experts 1,7 stay bf16 so the overall rel-err
stays ~1.8e-2 (< 2e-2 gate). The routing weight r_e and a 256x range scale
are folded into the weight cast (ACT, scale=256*r_e), and 256*r_e*be[e] is
preloaded into the PSUM bank by ACT before the matmul group (start=False
accumulates on top), so each expert epilogue is a single DVE op:
    acc += relu(psum)            # psum = 256*r_e*(x@We[e] + be[e])
acc holds 256*out until a final ACT descale before the store.

Experts 0/1 are cast before routing is known: e0 (fp8, rides phase A at a
6-tile lag behind the x transposes) uses unscaled weights and is rescaled
by r0 on ACT during e1; e1 (bf16) uses a 2-op DVE epilogue with r1 applied
at DVE time. e1's slow cadence + 5 rotating PSUM banks absorb the router
latency without stalling the PE.

DMA rings: x on sync+gpsimd, We[0] split scalar/vector, Wr1 trails on
sync/gpsimd, We[e>=2] alternate scalar/vector, be + router smalls on the
tensor ring (issued at kernel start, dependency-free), output on sync.
"""
import numpy as np
from contextlib import ExitStack

import concourse.bass as bass
import concourse.tile as tile
from concourse import bacc, mybir
from concourse.bass_utils import run_bass_kernel_spmd
from concourse.masks import make_identity

B, S, H, E = 8, 2048, 1024, 8
N_CORES = 8
P = 128
NF = 512
F32 = mybir.dt.float32
BF16 = mybir.dt.bfloat16
F8 = mybir.dt.float8e4
DR = mybir.MatmulPerfMode.DoubleRow
AX = mybir.AxisListType
ALU = mybir.AluOpType
ACTF = mybir.ActivationFunctionType

WS = 256.0  # range scale folded into every weight cast; acc = WS * out


def build_nc(s=S, dbg=False):
    t_tiles = s // P
    h_tiles = H // P
    hp = h_tiles // 2

    nc = bacc.Bacc("TRN2", target_bir_lowering=False, debug=False)
    x_ap = nc.dram_tensor("x", [s, H], BF16, kind="ExternalInput").ap()
    we_ap = nc.dram_tensor("We", [E, H, H], BF16, kind="ExternalInput").ap()
    be_ap = nc.dram_tensor("be", [E, H], F32, kind="ExternalInput").ap()
    wr1_ap = nc.dram_tensor("Wr1", [H, H], BF16, kind="ExternalInput").ap()
    br1_ap = nc.dram_tensor("br1", [H], F32, kind="ExternalInput").ap()
    wr2_ap = nc.dram_tensor("Wr2", [H, E], F32, kind="ExternalInput").ap()
    br2_ap = nc.dram_tensor("br2", [E], F32, kind="ExternalInput").ap()
    out_ap = nc.dram_tensor("out", [s, H], F32, kind="ExternalOutput").ap()
    if dbg:
        dbg_rsb = nc.dram_tensor("dbg_rsb", [P, E], F32, kind="ExternalOutput").ap()
        dbg_xm = nc.dram_tensor("dbg_xm", [1, H], BF16, kind="ExternalOutput").ap()
        dbg_xt8 = nc.dram_tensor("dbg_xt8", [P, H], F8, kind="ExternalOutput").ap()
        dbg_wq2 = nc.dram_tensor("dbg_wq2", [P, H], F8, kind="ExternalOutput").ap()
        dbg_acc0 = nc.dram_tensor("dbg_acc0", [P, H], F32, kind="ExternalOutput").ap()
        dbg_acce0 = nc.dram_tensor("dbg_acce0", [P, H], F32, kind="ExternalOutput").ap()
        dbg_wq1 = nc.dram_tensor("dbg_wq1", [P, H], BF16, kind="ExternalOutput").ap()
        dbg_wq0 = nc.dram_tensor("dbg_wq0", [P, H], F8, kind="ExternalOutput").ap()

    with tile.TileContext(nc) as tc, ExitStack() as ctx:
        xstage = ctx.enter_context(tc.tile_pool(name="xstage", bufs=2))
        xtpool = ctx.enter_context(tc.tile_pool(name="xt", bufs=1))
        accpool = ctx.enter_context(tc.tile_pool(name="acc", bufs=1))
        wrawp = ctx.enter_context(tc.tile_pool(name="wraw", bufs=5))
        wq8pool = ctx.enter_context(tc.tile_pool(name="wq8", bufs=2))
        wqpool = ctx.enter_context(tc.tile_pool(name="wq", bufs=2))
        bepool = ctx.enter_context(tc.tile_pool(name="bep", bufs=2))
        mpool = ctx.enter_context(tc.tile_pool(name="mp", bufs=2))
        rpool = ctx.enter_context(tc.tile_pool(name="rp", bufs=1))
        ps = ctx.enter_context(tc.tile_pool(name="ps", bufs=4, space="PSUM"))
        ps_t = ctx.enter_context(tc.tile_pool(name="ps_t", bufs=2, space="PSUM"))
        ps_x = ctx.enter_context(tc.tile_pool(name="ps_x", bufs=1, space="PSUM"))
        ps_r = ctx.enter_context(tc.tile_pool(name="ps_r", bufs=1, space="PSUM"))

        # ---- staged weight loading helper (DMA ring + ACT cast) ----
        class Loader:
            def __init__(self, name, src_fn, dst, scale_fn, qs, eng=None):
                self.name, self.src_fn, self.dst = name, src_fn, dst
                self.scale_fn, self.qs = scale_fn, qs
                self.eng = eng
                self.raws, self.done = {}, set()

            def dma(self, i):
                if i < h_tiles and i not in self.raws and i not in self.done:
                    t = wrawp.tile([P, H], BF16, tag="wraw", name=f"{self.name}r{i}")
                    self.qs[i % len(self.qs)].dma_start(t[:], self.src_fn(i))
                    self.raws[i] = t

            def cast(self, i):
                if i < h_tiles and i not in self.done:
                    self.dma(i)
                    if self.eng is None:
                        nc.vector.tensor_scalar_mul(
                            self.dst[:, i, :], self.raws.pop(i)[:], self.scale_fn()
                        )
                    else:
                        nc.scalar.mul(
                            self.dst[:, i, :], self.raws.pop(i)[:], self.scale_fn()
                        )
                    self.done.add(i)

            def flush(self):
                for i in range(h_tiles):
                    self.dma(i)
                for i in range(h_tiles):
                    self.cast(i)

        ones_row = rpool.tile([1, P], F32, tag="ones_row")
        nc.vector.memset(ones_row, 1.0)
        ones_col = rpool.tile([P, 1], BF16, tag="ones_col")
        nc.vector.memset(ones_col, 1.0)
        ident = rpool.tile([P, P], BF16, tag="ident")
        make_identity(nc, ident)

        # small router inputs on the tensor ring (issued at kernel start,
        # dependency-free -> never blocks the PE instruction stream)
        br1t = rpool.tile([P, h_tiles], F32, tag="br1t")
        nc.scalar.dma_start(br1t[:], br1_ap.rearrange("(d p) -> p d", p=P))
        w2r = rpool.tile([P, h_tiles, E], F32, tag="w2r")
        for dj in range(h_tiles):
            nc.scalar.dma_start(w2r[:, dj, :], wr2_ap[bass.ts(dj, P), :])
        br2t = rpool.tile([1, E], F32, tag="br2t")
        nc.scalar.dma_start(br2t[:], br2_ap.rearrange("(a e) -> a e", a=1))
        ber0 = bepool.tile([P, H], F32, tag="ber", name="ber0")
        nc.gpsimd.dma_start(ber0[:], be_ap[0:1, :].to_broadcast([P, H]))
        w2b = rpool.tile([P, h_tiles, E], BF16, tag="w2b")
        nc.scalar.copy(w2b[:], w2r[:])
        nc.scalar.mul(ber0[:], ber0[:], WS)  # in place: 256*be0

        # persistent SBUF residents
        xT = xtpool.tile([P, h_tiles, s], BF16, tag="xT")
        xT8 = xtpool.tile([P, h_tiles, s], F8, tag="xT8")
        acc = [accpool.tile([P, H], F32, tag=f"acc{i}", name=f"acc{i}") for i in range(t_tiles)]

        wq8_0 = wq8pool.tile([P, h_tiles, H], F8, tag="wq8", name="wq8_0")
        wr1b = wqpool.tile([P, h_tiles, H], BF16, tag="wq", name="wr1b")
        wq1 = wqpool.tile([P, h_tiles, H], BF16, tag="wq", name="wq1")

        ld_we0 = Loader(
            "we0", lambda i: we_ap[0, bass.ts(i, P), :], wq8_0,
            lambda: WS, [nc.scalar],
        )
        ld_we1 = Loader(
            "we1", lambda i: we_ap[1, bass.ts(i, P), :], wq1,
            lambda: WS, [nc.scalar],
        )
        class Wr1Direct:
            def __init__(self):
                self.done = set()

            def dma(self, i):
                if i < h_tiles and i not in self.done:
                    q = nc.sync if i % 2 == 0 else nc.gpsimd
                    q.dma_start(wr1b[:, i, :], wr1_ap[bass.ts(i, P), :])
                    self.done.add(i)

            def flush(self):
                for i in range(h_tiles):
                    self.dma(i)

        ld_wr1 = Wr1Direct()

        def emit_e0(k):
            p0 = ps.tile([P, NF], F32, tag="ps")
            p1 = ps.tile([P, NF], F32, tag="ps")
            nc.scalar.copy(p0[:], ber0[:, 0:NF])
            nc.scalar.copy(p1[:], ber0[:, NF:H])
            for jp in range(hp):
                lhs = xT8[:, 2 * jp : 2 * jp + 2, bass.ts(k, P)]
                nc.tensor.matmul(
                    p0[:], lhs, wq8_0[:, 2 * jp : 2 * jp + 2, 0:NF],
                    start=False, stop=(jp == hp - 1), perf_mode=DR,
                    skip_group_check=True,
                )
                nc.tensor.matmul(
                    p1[:], lhs, wq8_0[:, 2 * jp : 2 * jp + 2, NF:H],
                    start=False, stop=(jp == hp - 1), perf_mode=DR,
                    skip_group_check=True,
                )
            nc.vector.tensor_scalar_max(acc[k][:, 0:NF], p0[:], 0.0)
            nc.vector.tensor_scalar_max(acc[k][:, NF:H], p1[:], 0.0)

        xs_ps = ps_x.tile([33, NF], F32, tag="xs")

        # ---- phase A: stream x, transpose, expert 0 at a 6-tile lag ----
        E0_LAG = 99  # diagnostic: e0 fully after phase A
        for ti in range(t_tiles):
            q = nc.sync if ti % 2 == 0 else nc.gpsimd
            xs = xstage.tile([P, H], BF16, tag="xs")
            q.dma_start(xs[:], x_ap[bass.ts(ti, P), :])
            xb = xs
            for g in range(2):
                pt = ps_t.tile([P, 4, P], BF16, tag="pt")
                for k in range(4):
                    hj = 4 * g + k
                    nc.tensor.transpose(pt[:, k, :], xb[:, bass.ts(hj, P)], ident[:])
                nc.vector.tensor_copy(xT[:, 4 * g : 4 * g + 4, bass.ts(ti, P)], pt[:])
                nc.scalar.copy(xT8[:, 4 * g : 4 * g + 4, bass.ts(ti, P)], pt[:])
            nc.tensor.matmul(
                xs_ps[0:1, :], ones_col[:], xb[:, 0:NF],
                start=(ti == 0), stop=(ti == t_tiles - 1),
            )
            nc.tensor.matmul(
                xs_ps[32:33, :], ones_col[:], xb[:, NF:H],
                start=(ti == 0), stop=(ti == t_tiles - 1),
            )
            if ti < 4:
                ld_we0.dma(ti)
                ld_we0.dma(ti + 4)
            if ti >= 2 and ti % 2 == 0:
                ld_we0.cast((ti - 2) // 2)
            if ti >= 4:
                ld_we1.dma(ti - 4)
            if 8 <= ti < 12:
                ld_wr1.dma(ti - 8)
                ld_wr1.dma(ti - 4)  # pairs on alternating rings
            if ti >= E0_LAG:
                emit_e0(ti - E0_LAG)
        ld_we0.flush()
        for i in range(h_tiles):
            ld_wr1.dma(i)
        for k in range(max(t_tiles - E0_LAG, 0), t_tiles):
            emit_e0(k)
        ld_we1.flush()
        if dbg:
            nc.sync.dma_start(dbg_acce0, acc[min(2, t_tiles - 1)][:])
            nc.sync.dma_start(dbg_wq1, wq1[:, 0, :])
            nc.sync.dma_start(dbg_wq0, wq8_0[:, 6, :])

        # ---- router inputs: xsum accumulated in PSUM during phase A ----
        xmrb = rpool.tile([1, H], BF16, tag="xmrb")
        nc.scalar.mul(xmrb[:, 0:NF], xs_ps[0:1, :], 1.0 / s)
        nc.scalar.mul(xmrb[:, NF:H], xs_ps[32:33, :], 1.0 / s)
        xmT_ps = ps_r.tile([P, h_tiles, 2], BF16, tag="psr", name="xmT_ps")
        for j in range(h_tiles):
            nc.tensor.transpose(
                xmT_ps[:, j, 0:1], xmrb[:, bass.ts(j, P)], ident[0:1, 0:1]
            )
        xmeanb = rpool.tile([P, h_tiles], BF16, tag="xmeanb")
        for j in range(h_tiles):
            nc.scalar.copy(xmeanb[:, j : j + 1], xmT_ps[:, j, 0:1])
        ld_wr1.flush()

        # ---- expert 1 prefix: 2 tiles of bf16 matmuls cover router latency
        def e1_mms(ti):
            p0 = ps.tile([P, NF], F32, tag="ps")
            p1 = ps.tile([P, NF], F32, tag="ps")
            for hj in range(h_tiles):
                lhs = xT[:, hj, bass.ts(ti, P)]
                nc.tensor.matmul(
                    p0[:], lhs, wq1[:, hj, 0:NF],
                    start=(hj == 0), stop=(hj == h_tiles - 1),
                )
                nc.tensor.matmul(
                    p1[:], lhs, wq1[:, hj, NF:H],
                    start=(hj == 0), stop=(hj == h_tiles - 1),
                )
            return p0, p1

        PREFIX = min(2, t_tiles)
        prefix_ps = [(ti,) + e1_mms(ti) for ti in range(PREFIX)]

        # ---- router compute (PE bits interleave with e1's matmul stream)
        hvec_ps = ps_r.tile([P, h_tiles], F32, tag="psr", name="hvec_ps")
        for dj in range(h_tiles):
            for hj in range(h_tiles):
                nc.tensor.matmul(
                    hvec_ps[:, dj : dj + 1],
                    wr1b[:, hj, bass.ts(dj, P)],
                    xmeanb[:, hj : hj + 1],
                    start=(hj == 0),
                    stop=(hj == h_tiles - 1),
                )
        hsb = rpool.tile([P, h_tiles], F32, tag="hsb")
        nc.vector.tensor_add(hsb[:], hvec_ps[:], br1t[:])
        nc.vector.tensor_scalar_max(hsb[:], hsb[:], 0.0)
        hsbb = rpool.tile([P, h_tiles], BF16, tag="hsbb")
        nc.scalar.copy(hsbb[:], hsb[:])
        lg_ps = ps_r.tile([1, E], F32, tag="psr", name="lg_ps")
        for dj in range(h_tiles):
            nc.tensor.matmul(
                lg_ps[:], hsbb[:, dj : dj + 1], w2b[:, dj, :],
                start=(dj == 0), stop=(dj == h_tiles - 1),
            )
        logits = rpool.tile([1, E], F32, tag="logits")
        nc.vector.tensor_add(logits[:], lg_ps[:], br2t[:])
        mx = rpool.tile([1, 1], F32, tag="mx")
        nc.vector.reduce_max(mx[:], logits[:], axis=AX.X)
        nmx = rpool.tile([1, 1], F32, tag="nmx")
        nc.vector.tensor_scalar_mul(nmx[:], mx[:], -1.0)
        ex = rpool.tile([1, E], F32, tag="ex")
        nc.scalar.activation(ex[:], logits[:], ACTF.Exp, bias=nmx[:], scale=1.0)
        sm = rpool.tile([1, 1], F32, tag="sm")
        nc.vector.reduce_sum(sm[:], ex[:], axis=AX.X)
        rinv = rpool.tile([1, 1], F32, tag="rinv")
        nc.vector.reciprocal(rinv[:], sm[:])
        rvec = rpool.tile([1, E], F32, tag="rvec")
        nc.vector.tensor_scalar_mul(rvec[:], ex[:], rinv[:])
        r_ps = ps_r.tile([P, E], F32, tag="psr", name="r_ps")
        nc.tensor.matmul(r_ps[:], ones_row[:], rvec[:], start=True, stop=True)
        rsb = rpool.tile([P, E], F32, tag="rsb")
        nc.scalar.copy(rsb[:], r_ps[:])
        rsb256 = rpool.tile([P, E], F32, tag="rsb256")
        nc.scalar.mul(rsb256[:], rsb[:], WS)
        if dbg:
            nc.sync.dma_start(dbg_rsb, rsb[:])
            nc.sync.dma_start(dbg_xm, xmrb[:])
            for _j in range(h_tiles):
                nc.sync.dma_start(dbg_xt8[:, bass.ts(_j, P)], xT8[:, _j, bass.ts(min(2, t_tiles - 1), P)])

        # ---- expert 1 epilogues + remaining tiles ----
        ber1 = bepool.tile([P, H], F32, tag="ber", name="ber1")
        nc.gpsimd.dma_start(ber1[:], be_ap[1:2, :].to_broadcast([P, H]))
        bep1 = bepool.tile([P, H], F32, tag="ber", name="bep1")
        nc.scalar.mul(bep1[:], ber1[:], rsb256[:, 1:2])

        wq8_2 = wq8pool.tile([P, h_tiles, H], F8, tag="wq8", name="wq8_2")
        ld_we2 = Loader(
            "we2", lambda i: we_ap[2, bass.ts(i, P), :], wq8_2,
            lambda: rsb256[:, 2:3], [nc.scalar],
        )
        for i in range(4):
            ld_we2.dma(i)

        def e1_epis(ti, p0, p1):
            # deferred expert-0 routing weight, then e1's 2-op epilogue
            nc.scalar.mul(acc[ti][:], acc[ti][:], rsb[:, 0:1])
            for dc, pp in ((0, p0), (1, p1)):
                mt = mpool.tile([P, NF], F32, tag="mt")
                nc.vector.scalar_tensor_tensor(
                    mt[:], pp[:], rsb[:, 1:2], bep1[:, bass.ts(dc, NF)],
                    op0=ALU.mult, op1=ALU.add,
                )
                asl = acc[ti][:, bass.ts(dc, NF)]
                nc.vector.scalar_tensor_tensor(
                    asl, mt[:], 0.0, asl, op0=ALU.max, op1=ALU.add
                )

        for ti, p0, p1 in prefix_ps:
            e1_epis(ti, p0, p1)
        for ti in range(PREFIX, t_tiles):
            p0, p1 = e1_mms(ti)
            e1_epis(ti, p0, p1)
            if 4 <= ti < 8:
                ld_we2.dma(ti)
            if ti >= 5:
                ld_we2.cast(ti - 5)
        ld_we2.flush()
        if dbg:
            nc.sync.dma_start(dbg_wq2, wq8_2[:, 0, :])
            nc.sync.dma_start(dbg_acc0, acc[min(2, t_tiles - 1)][:])

        # ---- experts 2..7 ----
        wq_cur = wq8_2
        ld_next = None
        for e in range(2, E):
            is_f8 = e < E - 1
            ber = bepool.tile([P, H], F32, tag="ber", name=f"ber{e}")
            nc.gpsimd.dma_start(ber[:], be_ap[e : e + 1, :].to_broadcast([P, H]))
            nc.scalar.mul(ber[:], ber[:], rsb256[:, e : e + 1])  # 256*r_e*be

            ne = e + 1
            if ne < E:
                if ne < E - 1:
                    wq_next = wq8pool.tile(
                        [P, h_tiles, H], F8, tag="wq8", name=f"wq8_{ne}"
                    )
                else:
                    wq_next = wqpool.tile(
                        [P, h_tiles, H], BF16, tag="wq", name=f"wq{ne}"
                    )
                nq = nc.scalar if ne % 2 == 0 else nc.gpsimd
                ld_next = Loader(
                    f"we{ne}", lambda i, _e=ne: we_ap[_e, bass.ts(i, P), :],
                    wq_next, lambda _e=ne: rsb256[:, _e : _e + 1], [nq],
                )

            for ti in range(t_tiles):
                p0 = ps.tile([P, NF], F32, tag="ps")
                p1 = ps.tile([P, NF], F32, tag="ps")
                nc.scalar.copy(p0[:], ber[:, 0:NF])
                nc.scalar.copy(p1[:], ber[:, NF:H])
                if is_f8:
                    for jp in range(hp):
                        lhs = xT8[:, 2 * jp : 2 * jp + 2, bass.ts(ti, P)]
                        nc.tensor.matmul(
                            p0[:], lhs, wq_cur[:, 2 * jp : 2 * jp + 2, 0:NF],
                            start=False, stop=(jp == hp - 1), perf_mode=DR,
                            skip_group_check=True,
                        )
                        nc.tensor.matmul(
                            p1[:], lhs, wq_cur[:, 2 * jp : 2 * jp + 2, NF:H],
                            start=False, stop=(jp == hp - 1), perf_mode=DR,
                            skip_group_check=True,
                        )
                else:
                    for hj in range(h_tiles):
                        lhs = xT[:, hj, bass.ts(ti, P)]
                        nc.tensor.matmul(
                            p0[:], lhs, wq_cur[:, hj, 0:NF],
                            start=False, stop=(hj == h_tiles - 1),
                            skip_group_check=True,
                        )
                        nc.tensor.matmul(
                            p1[:], lhs, wq_cur[:, hj, NF:H],
                            start=False, stop=(hj == h_tiles - 1),
                            skip_group_check=True,
                        )
                for dc, pp in ((0, p0), (1, p1)):
                    asl = acc[ti][:, bass.ts(dc, NF)]
                    nc.vector.scalar_tensor_tensor(
                        asl, pp[:], 0.0, asl, op0=ALU.max, op1=ALU.add
                    )
                if e == E - 1:
                    nc.vector.tensor_scalar_mul(acc[ti][:], acc[ti][:], 1.0 / WS)
                    nc.sync.dma_start(out_ap[bass.ts(ti, P), :], acc[ti][:])
                elif ti % 2 == 0:
                    ld_next.dma(ti // 2)
                    if ti >= 2:
                        ld_next.cast(ti // 2 - 1)
            if ne < E:
                ld_next.flush()
                wq_cur = wq_next

    nc.compile()
    return nc


_nc_cache = {}


def _get_nc(s):
    if s not in _nc_cache:
        _nc_cache[s] = build_nc(s)
    return _nc_cache[s]


def kernel(x, We, be, Wr1, br1, Wr2, br2):
    import ml_dtypes

    x = np.ascontiguousarray(np.asarray(x, dtype=np.float32).astype(ml_dtypes.bfloat16))
    We = np.ascontiguousarray(np.asarray(We, dtype=np.float32).astype(ml_dtypes.bfloat16))
    be = np.ascontiguousarray(np.asarray(be, dtype=np.float32))
    Wr1 = np.ascontiguousarray(np.asarray(Wr1, dtype=np.float32).astype(ml_dtypes.bfloat16))
    br1 = np.ascontiguousarray(np.asarray(br1, dtype=np.float32))
    Wr2 = np.ascontiguousarray(np.asarray(Wr2, dtype=np.float32))
    br2 = np.ascontiguousarray(np.asarray(br2, dtype=np.float32))

    s = x.shape[1]
    nc = _get_nc(s)
    shared = {"We": We, "be": be, "Wr1": Wr1, "br1": br1, "Wr2": Wr2, "br2": br2}
    in_maps = [{"x": x[c], **shared} for c in range(N_CORES)]
    res = run_bass_kernel_spmd(nc, in_maps, list(range(N_CORES)))
    return np.stack([res.results[c]["out"] for c in range(N_CORES)], axis=0)


# revision 29
# speedup vs baseline: 1.0051x; 1.0051x over previous
"""Trainium2 Bass kernel for nn_KVCacheMoE (B=8, S=2048, H=1024, E=8).

Batch-parallel across the 8 NeuronCores (core c owns batch c) - the router
depends only on that batch, so no collectives.

Experts 0,2,3,4,5,6 run in fp8-e4m3 with perf_mode=DoubleRow (2.0x PE
throughput measured vs bf16); experts 1,7 stay bf16 so the overall rel-err
stays ~1.8e-2 (< 2e-2 gate). The routing weight r_e and a 256x range scale
are folded into the weight cast (ACT, scale=256*r_e), and 256*r_e*be[e] is
preloaded into the PSUM bank by ACT before the matmul group (start=False
accumulates on top), so each expert epilogue is a single DVE op:
    acc += relu(psum)            # psum = 256*r_e*(x@We[e] + be[e])
acc holds 256*out until a final ACT descale before the store.

Experts 0/1 are cast before routing is known: e0 (fp8, rides phase A at a
6-tile lag behind the x transposes) uses unscaled weights and is rescaled
by r0 on ACT during e1; e1 (bf16) uses a 2-op DVE epilogue with r1 applied
at DVE time. e1's slow cadence + 5 rotating PSUM banks absorb the router
latency without stalling the PE.

DMA rings: x on sync+gpsimd, We[0] split scalar/vector, Wr1 trails on
sync/gpsimd, We[e>=2] alternate scalar/vector, be + router smalls on the
tensor ring (issued at kernel start, dependency-free), output on sync.
"""
import numpy as np
from contextlib import ExitStack

import concourse.bass as bass
import concourse.tile as tile
from concourse import bacc, mybir
from concourse.bass_utils import run_bass_kernel_spmd
from concourse.masks import make_identity

B, S, H, E = 8, 2048, 1024, 8
N_CORES = 8
P = 128
NF = 512
F32 = mybir.dt.float32
BF16 = mybir.dt.bfloat16
F8 = mybir.dt.float8e4
DR = mybir.MatmulPerfMode.DoubleRow
AX = mybir.AxisListType
ALU = mybir.AluOpType
ACTF = mybir.ActivationFunctionType

WS = 256.0  # range scale folded into every weight cast; acc = WS * out


def build_nc(s=S, dbg=False):
    t_tiles = s // P
    h_tiles = H // P
    hp = h_tiles // 2

    nc = bacc.Bacc("TRN2", target_bir_lowering=False, debug=False)
    x_ap = nc.dram_tensor("x", [s, H], BF16, kind="ExternalInput").ap()
    we_ap = nc.dram_tensor("We", [E, H, H], BF16, kind="ExternalInput").ap()
    be_ap = nc.dram_tensor("be", [E, H], F32, kind="ExternalInput").ap()
    wr1_ap = nc.dram_tensor("Wr1", [H, H], BF16, kind="ExternalInput").ap()
    br1_ap = nc.dram_tensor("br1", [H], F32, kind="ExternalInput").ap()
    wr2_ap = nc.dram_tensor("Wr2", [H, E], F32, kind="ExternalInput").ap()
    br2_ap = nc.dram_tensor("br2", [E], F32, kind="ExternalInput").ap()
    out_ap = nc.dram_tensor("out", [s, H], F32, kind="ExternalOutput").ap()
    if dbg:
        dbg_rsb = nc.dram_tensor("dbg_rsb", [P, E], F32, kind="ExternalOutput").ap()
        dbg_xm = nc.dram_tensor("dbg_xm", [1, H], BF16, kind="ExternalOutput").ap()
        dbg_xt8 = nc.dram_tensor("dbg_xt8", [P, H], F8, kind="ExternalOutput").ap()
        dbg_wq2 = nc.dram_tensor("dbg_wq2", [P, H], F8, kind="ExternalOutput").ap()
        dbg_acc0 = nc.dram_tensor("dbg_acc0", [P, H], F32, kind="ExternalOutput").ap()
        dbg_acce0 = nc.dram_tensor("dbg_acce0", [P, H], F32, kind="ExternalOutput").ap()
        dbg_wq1 = nc.dram_tensor("dbg_wq1", [P, H], BF16, kind="ExternalOutput").ap()
        dbg_wq0 = nc.dram_tensor("dbg_wq0", [P, H], F8, kind="ExternalOutput").ap()

    with tile.TileContext(nc) as tc, ExitStack() as ctx:
        xstage = ctx.enter_context(tc.tile_pool(name="xstage", bufs=2))
        xtpool = ctx.enter_context(tc.tile_pool(name="xt", bufs=1))
        accpool = ctx.enter_context(tc.tile_pool(name="acc", bufs=1))
        wrawp = ctx.enter_context(tc.tile_pool(name="wraw", bufs=5))
        wq8pool = ctx.enter_context(tc.tile_pool(name="wq8", bufs=2))
        wqpool = ctx.enter_context(tc.tile_pool(name="wq", bufs=2))
        bepool = ctx.enter_context(tc.tile_pool(name="bep", bufs=2))
        mpool = ctx.enter_context(tc.tile_pool(name="mp", bufs=2))
        rpool = ctx.enter_context(tc.tile_pool(name="rp", bufs=1))
        ps = ctx.enter_context(tc.tile_pool(name="ps", bufs=4, space="PSUM"))
        ps_t = ctx.enter_context(tc.tile_pool(name="ps_t", bufs=2, space="PSUM"))
        ps_x = ctx.enter_context(tc.tile_pool(name="ps_x", bufs=1, space="PSUM"))
        ps_r = ctx.enter_context(tc.tile_pool(name="ps_r", bufs=1, space="PSUM"))

        # ---- staged weight loading helper (DMA ring + ACT cast) ----
        class Loader:
            def __init__(self, name, src_fn, dst, scale_fn, qs, eng=None):
                self.name, self.src_fn, self.dst = name, src_fn, dst
                self.scale_fn, self.qs = scale_fn, qs
                self.eng = eng
                self.raws, self.done = {}, set()

            def dma(self, i):
                if i < h_tiles and i not in self.raws and i not in self.done:
                    t = wrawp.tile([P, H], BF16, tag="wraw", name=f"{self.name}r{i}")
                    self.qs[i % len(self.qs)].dma_start(t[:], self.src_fn(i))
                    self.raws[i] = t

            def cast(self, i):
                if i < h_tiles and i not in self.done:
                    self.dma(i)
                    if self.eng is None:
                        nc.vector.tensor_scalar_mul(
                            self.dst[:, i, :], self.raws.pop(i)[:], self.scale_fn()
                        )
                    else:
                        nc.scalar.mul(
                            self.dst[:, i, :], self.raws.pop(i)[:], self.scale_fn()
                        )
                    self.done.add(i)

            def flush(self):
                for i in range(h_tiles):
                    self.dma(i)
                for i in range(h_tiles):
                    self.cast(i)

        ones_row = rpool.tile([1, P], F32, tag="ones_row")
        nc.vector.memset(ones_row, 1.0)
        ones_col = rpool.tile([P, 1], BF16, tag="ones_col")
        nc.vector.memset(ones_col, 1.0)
        ident = rpool.tile([P, P], BF16, tag="ident")
        make_identity(nc, ident)

        # small router inputs on the tensor ring (issued at kernel start,
        # dependency-free -> never blocks the PE instruction stream)
        br1t = rpool.tile([P, h_tiles], F32, tag="br1t")
        nc.scalar.dma_start(br1t[:], br1_ap.rearrange("(d p) -> p d", p=P))
        w2r = rpool.tile([P, h_tiles, E], F32, tag="w2r")
        for dj in range(h_tiles):
            nc.scalar.dma_start(w2r[:, dj, :], wr2_ap[bass.ts(dj, P), :])
        br2t = rpool.tile([1, E], F32, tag="br2t")
        nc.scalar.dma_start(br2t[:], br2_ap.rearrange("(a e) -> a e", a=1))
        ber0 = bepool.tile([P, H], F32, tag="ber", name="ber0")
        nc.gpsimd.dma_start(ber0[:], be_ap[0:1, :].to_broadcast([P, H]))
        w2b = rpool.tile([P, h_tiles, E], BF16, tag="w2b")
        nc.scalar.copy(w2b[:], w2r[:])
        nc.scalar.mul(ber0[:], ber0[:], WS)  # in place: 256*be0

        # persistent SBUF residents
        xT = xtpool.tile([P, h_tiles, s], BF16, tag="xT")
        xT8 = xtpool.tile([P, h_tiles, s], F8, tag="xT8")
        acc = [accpool.tile([P, H], F32, tag=f"acc{i}", name=f"acc{i}") for i in range(t_tiles)]

        wq8_0 = wq8pool.tile([P, h_tiles, H], F8, tag="wq8", name="wq8_0")
        wr1b = wqpool.tile([P, h_tiles, H], BF16, tag="wq", name="wr1b")
        wq1 = wqpool.tile([P, h_tiles, H], BF16, tag="wq", name="wq1")

        ld_we0 = Loader(
            "we0", lambda i: we_ap[0, bass.ts(i, P), :], wq8_0,
            lambda: WS, [nc.scalar],
        )
        ld_we1 = Loader(
            "we1", lambda i: we_ap[1, bass.ts(i, P), :], wq1,
            lambda: WS, [nc.scalar],
        )
        class Wr1Direct:
            def __init__(self):
                self.done = set()

            def dma(self, i):
                if i < h_tiles and i not in self.done:
                    q = nc.sync if i % 2 == 0 else nc.gpsimd
                    q.dma_start(wr1b[:, i, :], wr1_ap[bass.ts(i, P), :])
                    self.done.add(i)

            def flush(self):
                for i in range(h_tiles):
                    self.dma(i)

        ld_wr1 = Wr1Direct()

        def emit_e0(k):
            p0 = ps.tile([P, NF], F32, tag="ps")
            p1 = ps.tile([P, NF], F32, tag="ps")
            nc.scalar.copy(p0[:], ber0[:, 0:NF])
            nc.scalar.copy(p1[:], ber0[:, NF:H])
            for jp in range(hp):
                lhs = xT8[:, 2 * jp : 2 * jp + 2, bass.ts(k, P)]
                nc.tensor.matmul(
                    p0[:], lhs, wq8_0[:, 2 * jp : 2 * jp + 2, 0:NF],
                    start=False, stop=(jp == hp - 1), perf_mode=DR,
                    skip_group_check=True,
                )
                nc.tensor.matmul(
                    p1[:], lhs, wq8_0[:, 2 * jp : 2 * jp + 2, NF:H],
                    start=False, stop=(jp == hp - 1), perf_mode=DR,
                    skip_group_check=True,
                )
            nc.vector.tensor_scalar_max(acc[k][:, 0:NF], p0[:], 0.0)
            nc.vector.tensor_scalar_max(acc[k][:, NF:H], p1[:], 0.0)

        xs_ps = ps_x.tile([33, NF], F32, tag="xs")

        # ---- phase A: stream x, transpose, expert 0 at a 6-tile lag ----
        E0_LAG = 99  # diagnostic: e0 fully after phase A
        for ti in range(t_tiles):
            q = nc.sync if ti % 2 == 0 else nc.gpsimd
            xs = xstage.tile([P, H], BF16, tag="xs")
            q.dma_start(xs[:], x_ap[bass.ts(ti, P), :])
            xb = xs
            for g in range(2):
                pt = ps_t.tile([P, 4, P], BF16, tag="pt")
                for k in range(4):
                    hj = 4 * g + k
                    nc.tensor.transpose(pt[:, k, :], xb[:, bass.ts(hj, P)], ident[:])
                nc.vector.tensor_copy(xT[:, 4 * g : 4 * g + 4, bass.ts(ti, P)], pt[:])
                nc.scalar.copy(xT8[:, 4 * g : 4 * g + 4, bass.ts(ti, P)], pt[:])
            nc.tensor.matmul(
                xs_ps[0:1, :], ones_col[:], xb[:, 0:NF],
                start=(ti == 0), stop=(ti == t_tiles - 1),
            )
            nc.tensor.matmul(
                xs_ps[32:33, :], ones_col[:], xb[:, NF:H],
                start=(ti == 0), stop=(ti == t_tiles - 1),
            )
            if ti < 4:
                ld_we0.dma(ti)
                ld_we0.dma(ti + 4)
            if ti >= 2:
                ld_we0.cast(ti - 2)
            if ti >= 4:
                ld_we1.dma(ti - 4)
            if ti >= 6:
                ld_we1.cast(ti - 6)
            if 8 <= ti < 12:
                ld_wr1.dma(ti - 8)
                ld_wr1.dma(ti - 4)  # pairs on alternating rings
            if ti >= E0_LAG:
                emit_e0(ti - E0_LAG)
        ld_we0.flush()
        ld_we1.flush()
        for i in range(h_tiles):
            ld_wr1.dma(i)
        for k in range(max(t_tiles - E0_LAG, 0), t_tiles):
            emit_e0(k)
        if dbg:
            nc.sync.dma_start(dbg_acce0, acc[min(2, t_tiles - 1)][:])
            nc.sync.dma_start(dbg_wq1, wq1[:, 0, :])
            nc.sync.dma_start(dbg_wq0, wq8_0[:, 6, :])

        # ---- router inputs: xsum accumulated in PSUM during phase A ----
        xmrb = rpool.tile([1, H], BF16, tag="xmrb")
        nc.scalar.mul(xmrb[:, 0:NF], xs_ps[0:1, :], 1.0 / s)
        nc.scalar.mul(xmrb[:, NF:H], xs_ps[32:33, :], 1.0 / s)
        xmT_ps = ps_r.tile([P, h_tiles, 2], BF16, tag="psr", name="xmT_ps")
        for j in range(h_tiles):
            nc.tensor.transpose(
                xmT_ps[:, j, 0:1], xmrb[:, bass.ts(j, P)], ident[0:1, 0:1]
            )
        xmeanb = rpool.tile([P, h_tiles], BF16, tag="xmeanb")
        for j in range(h_tiles):
            nc.scalar.copy(xmeanb[:, j : j + 1], xmT_ps[:, j, 0:1])
        ld_wr1.flush()

        # ---- expert 1 prefix: 2 tiles of bf16 matmuls cover router latency
        def e1_mms(ti):
            p0 = ps.tile([P, NF], F32, tag="ps")
            p1 = ps.tile([P, NF], F32, tag="ps")
            for hj in range(h_tiles):
                lhs = xT[:, hj, bass.ts(ti, P)]
                nc.tensor.matmul(
                    p0[:], lhs, wq1[:, hj, 0:NF],
                    start=(hj == 0), stop=(hj == h_tiles - 1),
                )
                nc.tensor.matmul(
                    p1[:], lhs, wq1[:, hj, NF:H],
                    start=(hj == 0), stop=(hj == h_tiles - 1),
                )
            return p0, p1

        PREFIX = min(2, t_tiles)
        prefix_ps = [(ti,) + e1_mms(ti) for ti in range(PREFIX)]

        # ---- router compute (PE bits interleave with e1's matmul stream)
        hvec_ps = ps_r.tile([P, h_tiles], F32, tag="psr", name="hvec_ps")
        for dj in range(h_tiles):
            for hj in range(h_tiles):
                nc.tensor.matmul(
                    hvec_ps[:, dj : dj + 1],
                    wr1b[:, hj, bass.ts(dj, P)],
                    xmeanb[:, hj : hj + 1],
                    start=(hj == 0),
                    stop=(hj == h_tiles - 1),
                )
        hsb = rpool.tile([P, h_tiles], F32, tag="hsb")
        nc.vector.tensor_add(hsb[:], hvec_ps[:], br1t[:])
        nc.vector.tensor_scalar_max(hsb[:], hsb[:], 0.0)
        hsbb = rpool.tile([P, h_tiles], BF16, tag="hsbb")
        nc.scalar.copy(hsbb[:], hsb[:])
        lg_ps = ps_r.tile([1, E], F32, tag="psr", name="lg_ps")
        for dj in range(h_tiles):
            nc.tensor.matmul(
                lg_ps[:], hsbb[:, dj : dj + 1], w2b[:, dj, :],
                start=(dj == 0), stop=(dj == h_tiles - 1),
            )
        logits = rpool.tile([1, E], F32, tag="logits")
        nc.vector.tensor_add(logits[:], lg_ps[:], br2t[:])
        mx = rpool.tile([1, 1], F32, tag="mx")
        nc.vector.reduce_max(mx[:], logits[:], axis=AX.X)
        nmx = rpool.tile([1, 1], F32, tag="nmx")
        nc.vector.tensor_scalar_mul(nmx[:], mx[:], -1.0)
        ex = rpool.tile([1, E], F32, tag="ex")
        nc.scalar.activation(ex[:], logits[:], ACTF.Exp, bias=nmx[:], scale=1.0)
        sm = rpool.tile([1, 1], F32, tag="sm")
        nc.vector.reduce_sum(sm[:], ex[:], axis=AX.X)
        rinv = rpool.tile([1, 1], F32, tag="rinv")
        nc.vector.reciprocal(rinv[:], sm[:])
        rvec = rpool.tile([1, E], F32, tag="rvec")
        nc.vector.tensor_scalar_mul(rvec[:], ex[:], rinv[:])
        r_ps = ps_r.tile([P, E], F32, tag="psr", name="r_ps")
        nc.tensor.matmul(r_ps[:], ones_row[:], rvec[:], start=True, stop=True)
        rsb = rpool.tile([P, E], F32, tag="rsb")
        nc.scalar.copy(rsb[:], r_ps[:])
        rsb256 = rpool.tile([P, E], F32, tag="rsb256")
        nc.scalar.mul(rsb256[:], rsb[:], WS)
        if dbg:
            nc.sync.dma_start(dbg_rsb, rsb[:])
            nc.sync.dma_start(dbg_xm, xmrb[:])
            for _j in range(h_tiles):
                nc.sync.dma_start(dbg_xt8[:, bass.ts(_j, P)], xT8[:, _j, bass.ts(min(2, t_tiles - 1), P)])

        # ---- expert 1 epilogues + remaining tiles ----
        ber1 = bepool.tile([P, H], F32, tag="ber", name="ber1")
        nc.gpsimd.dma_start(ber1[:], be_ap[1:2, :].to_broadcast([P, H]))
        bep1 = bepool.tile([P, H], F32, tag="ber", name="bep1")
        nc.scalar.mul(bep1[:], ber1[:], rsb256[:, 1:2])

        wq8_2 = wq8pool.tile([P, h_tiles, H], F8, tag="wq8", name="wq8_2")
        ld_we2 = Loader(
            "we2", lambda i: we_ap[2, bass.ts(i, P), :], wq8_2,
            lambda: rsb256[:, 2:3], [nc.scalar],
        )
        for i in range(4):
            ld_we2.dma(i)

        def e1_epis(ti, p0, p1):
            # deferred expert-0 routing weight, then e1's 2-op epilogue
            nc.scalar.mul(acc[ti][:], acc[ti][:], rsb[:, 0:1])
            for dc, pp in ((0, p0), (1, p1)):
                mt = mpool.tile([P, NF], F32, tag="mt")
                nc.vector.scalar_tensor_tensor(
                    mt[:], pp[:], rsb[:, 1:2], bep1[:, bass.ts(dc, NF)],
                    op0=ALU.mult, op1=ALU.add,
                )
                asl = acc[ti][:, bass.ts(dc, NF)]
                nc.vector.scalar_tensor_tensor(
                    asl, mt[:], 0.0, asl, op0=ALU.max, op1=ALU.add
                )

        for ti, p0, p1 in prefix_ps:
            e1_epis(ti, p0, p1)
        for ti in range(PREFIX, t_tiles):
            p0, p1 = e1_mms(ti)
            e1_epis(ti, p0, p1)
            if 4 <= ti < 8:
                ld_we2.dma(ti)
            if ti >= 5:
                ld_we2.cast(ti - 5)
        ld_we2.flush()
        if dbg:
            nc.sync.dma_start(dbg_wq2, wq8_2[:, 0, :])
            nc.sync.dma_start(dbg_acc0, acc[min(2, t_tiles - 1)][:])

        # ---- experts 2..7 ----
        wq_cur = wq8_2
        ld_next = None
        for e in range(2, E):
            is_f8 = e < E - 1
            ber = bepool.tile([P, H], F32, tag="ber", name=f"ber{e}")
            nc.gpsimd.dma_start(ber[:], be_ap[e : e + 1, :].to_broadcast([P, H]))
            nc.scalar.mul(ber[:], ber[:], rsb256[:, e : e + 1])  # 256*r_e*be

            ne = e + 1
            if ne < E:
                if ne < E - 1:
                    wq_next = wq8pool.tile(
                        [P, h_tiles, H], F8, tag="wq8", name=f"wq8_{ne}"
                    )
                else:
                    wq_next = wqpool.tile(
                        [P, h_tiles, H], BF16, tag="wq", name=f"wq{ne}"
                    )
                nq = nc.scalar if ne % 2 == 0 else nc.gpsimd
                ld_next = Loader(
                    f"we{ne}", lambda i, _e=ne: we_ap[_e, bass.ts(i, P), :],
                    wq_next, lambda _e=ne: rsb256[:, _e : _e + 1], [nq],
                )

            for ti in range(t_tiles):
                p0 = ps.tile([P, NF], F32, tag="ps")
                p1 = ps.tile([P, NF], F32, tag="ps")
                nc.scalar.copy(p0[:], ber[:, 0:NF])
                nc.scalar.copy(p1[:], ber[:, NF:H])
                if is_f8:
                    for jp in range(hp):
                        lhs = xT8[:, 2 * jp : 2 * jp + 2, bass.ts(ti, P)]
                        nc.tensor.matmul(
                            p0[:], lhs, wq_cur[:, 2 * jp : 2 * jp + 2, 0:NF],
                            start=False, stop=(jp == hp - 1), perf_mode=DR,
                            skip_group_check=True,
                        )
                        nc.tensor.matmul(
                            p1[:], lhs, wq_cur[:, 2 * jp : 2 * jp + 2, NF:H],
                            start=False, stop=(jp == hp - 1), perf_mode=DR,
                            skip_group_check=True,
                        )
                else:
                    for hj in range(h_tiles):
                        lhs = xT[:, hj, bass.ts(ti, P)]
                        nc.tensor.matmul(
                            p0[:], lhs, wq_cur[:, hj, 0:NF],
                            start=False, stop=(hj == h_tiles - 1),
                            skip_group_check=True,
                        )
                        nc.tensor.matmul(
                            p1[:], lhs, wq_cur[:, hj, NF:H],
                            start=False, stop=(hj == h_tiles - 1),
                            skip_group_check=True,
                        )
                for dc, pp in ((0, p0), (1, p1)):
                    asl = acc[ti][:, bass.ts(dc, NF)]
                    nc.vector.scalar_tensor_tensor(
                        asl, pp[:], 0.0, asl, op0=ALU.max, op1=ALU.add
                    )
                if e == E - 1:
                    nc.vector.tensor_scalar_mul(acc[ti][:], acc[ti][:], 1.0 / WS)
                    nc.sync.dma_start(out_ap[bass.ts(ti, P), :], acc[ti][:])
                elif ti % 2 == 0:
                    ld_next.dma(ti // 2)
                    if ti >= 2:
                        ld_next.cast(ti // 2 - 1)
            if ne < E:
                ld_next.flush()
                wq_cur = wq_next

    nc.compile()
    return nc


_nc_cache = {}


def _get_nc(s):
    if s not in _nc_cache:
        _nc_cache[s] = build_nc(s)
    return _nc_cache[s]


def kernel(x, We, be, Wr1, br1, Wr2, br2):
    import ml_dtypes

    x = np.ascontiguousarray(np.asarray(x, dtype=np.float32).astype(ml_dtypes.bfloat16))
    We = np.ascontiguousarray(np.asarray(We, dtype=np.float32).astype(ml_dtypes.bfloat16))
    be = np.ascontiguousarray(np.asarray(be, dtype=np.float32))
    Wr1 = np.ascontiguousarray(np.asarray(Wr1, dtype=np.float32).astype(ml_dtypes.bfloat16))
    br1 = np.ascontiguousarray(np.asarray(br1, dtype=np.float32))
    Wr2 = np.ascontiguousarray(np.asarray(Wr2, dtype=np.float32))
    br2 = np.ascontiguousarray(np.asarray(br2, dtype=np.float32))

    s = x.shape[1]
    nc = _get_nc(s)
    shared = {"We": We, "be": be, "Wr1": Wr1, "br1": br1, "Wr2": Wr2, "br2": br2}
    in_maps = [{"x": x[c], **shared} for c in range(N_CORES)]
    res = run_bass_kernel_spmd(nc, in_maps, list(range(N_CORES)))
    return np.stack([res.results[c]["out"] for c in range(N_CORES)], axis=0)


# revision 30
# speedup vs baseline: 1.0121x; 1.0069x over previous
"""Trainium2 Bass kernel for nn_KVCacheMoE (B=8, S=2048, H=1024, E=8).

Batch-parallel across the 8 NeuronCores (core c owns batch c) - the router
depends only on that batch, so no collectives.

Experts 0,2,3,4,5,6 run in fp8-e4m3 with perf_mode=DoubleRow (2.0x PE
throughput measured vs bf16); experts 1,7 stay bf16 so the overall rel-err
stays ~1.8e-2 (< 2e-2 gate). The routing weight r_e and a 256x range scale
are folded into the weight cast (ACT, scale=256*r_e), and 256*r_e*be[e] is
preloaded into the PSUM bank by ACT before the matmul group (start=False
accumulates on top), so each expert epilogue is a single DVE op:
    acc += relu(psum)            # psum = 256*r_e*(x@We[e] + be[e])
acc holds 256*out until a final ACT descale before the store.

Experts 0/1 are cast before routing is known: e0 (fp8, rides phase A at a
6-tile lag behind the x transposes) uses unscaled weights and is rescaled
by r0 on ACT during e1; e1 (bf16) uses a 2-op DVE epilogue with r1 applied
at DVE time. e1's slow cadence + 5 rotating PSUM banks absorb the router
latency without stalling the PE.

DMA rings: x on sync+gpsimd, We[0] split scalar/vector, Wr1 trails on
sync/gpsimd, We[e>=2] alternate scalar/vector, be + router smalls on the
tensor ring (issued at kernel start, dependency-free), output on sync.
"""
import numpy as np
from contextlib import ExitStack

import concourse.bass as bass
import concourse.tile as tile
from concourse import bacc, mybir
from concourse.bass_utils import run_bass_kernel_spmd
from concourse.masks import make_identity

B, S, H, E = 8, 2048, 1024, 8
N_CORES = 8
P = 128
NF = 512
F32 = mybir.dt.float32
BF16 = mybir.dt.bfloat16
F8 = mybir.dt.float8e4
DR = mybir.MatmulPerfMode.DoubleRow
AX = mybir.AxisListType
ALU = mybir.AluOpType
ACTF = mybir.ActivationFunctionType

WS = 256.0  # range scale folded into every weight cast; acc = WS * out


def build_nc(s=S, dbg=False):
    t_tiles = s // P
    h_tiles = H // P
    hp = h_tiles // 2

    nc = bacc.Bacc("TRN2", target_bir_lowering=False, debug=False)
    x_ap = nc.dram_tensor("x", [s, H], BF16, kind="ExternalInput").ap()
    we_ap = nc.dram_tensor("We", [E, H, H], BF16, kind="ExternalInput").ap()
    be_ap = nc.dram_tensor("be", [E, H], F32, kind="ExternalInput").ap()
    wr1_ap = nc.dram_tensor("Wr1", [H, H], BF16, kind="ExternalInput").ap()
    br1_ap = nc.dram_tensor("br1", [H], F32, kind="ExternalInput").ap()
    wr2_ap = nc.dram_tensor("Wr2", [H, E], F32, kind="ExternalInput").ap()
    br2_ap = nc.dram_tensor("br2", [E], F32, kind="ExternalInput").ap()
    out_ap = nc.dram_tensor("out", [s, H], F32, kind="ExternalOutput").ap()
    if dbg:
        dbg_rsb = nc.dram_tensor("dbg_rsb", [P, E], F32, kind="ExternalOutput").ap()
        dbg_xm = nc.dram_tensor("dbg_xm", [1, H], BF16, kind="ExternalOutput").ap()
        dbg_xt8 = nc.dram_tensor("dbg_xt8", [P, H], F8, kind="ExternalOutput").ap()
        dbg_wq2 = nc.dram_tensor("dbg_wq2", [P, H], F8, kind="ExternalOutput").ap()
        dbg_acc0 = nc.dram_tensor("dbg_acc0", [P, H], F32, kind="ExternalOutput").ap()
        dbg_acce0 = nc.dram_tensor("dbg_acce0", [P, H], F32, kind="ExternalOutput").ap()
        dbg_wq1 = nc.dram_tensor("dbg_wq1", [P, H], BF16, kind="ExternalOutput").ap()
        dbg_wq0 = nc.dram_tensor("dbg_wq0", [P, H], F8, kind="ExternalOutput").ap()

    with tile.TileContext(nc) as tc, ExitStack() as ctx:
        xstage = ctx.enter_context(tc.tile_pool(name="xstage", bufs=2))
        xtpool = ctx.enter_context(tc.tile_pool(name="xt", bufs=1))
        accpool = ctx.enter_context(tc.tile_pool(name="acc", bufs=1))
        wrawp = ctx.enter_context(tc.tile_pool(name="wraw", bufs=5))
        wq8pool = ctx.enter_context(tc.tile_pool(name="wq8", bufs=2))
        wqpool = ctx.enter_context(tc.tile_pool(name="wq", bufs=2))
        bepool = ctx.enter_context(tc.tile_pool(name="bep", bufs=2))
        mpool = ctx.enter_context(tc.tile_pool(name="mp", bufs=2))
        rpool = ctx.enter_context(tc.tile_pool(name="rp", bufs=1))
        ps = ctx.enter_context(tc.tile_pool(name="ps", bufs=4, space="PSUM"))
        ps_t = ctx.enter_context(tc.tile_pool(name="ps_t", bufs=2, space="PSUM"))
        ps_x = ctx.enter_context(tc.tile_pool(name="ps_x", bufs=1, space="PSUM"))
        ps_r = ctx.enter_context(tc.tile_pool(name="ps_r", bufs=1, space="PSUM"))

        # ---- staged weight loading helper (DMA ring + ACT cast) ----
        class Loader:
            def __init__(self, name, src_fn, dst, scale_fn, qs, eng=None):
                self.name, self.src_fn, self.dst = name, src_fn, dst
                self.scale_fn, self.qs = scale_fn, qs
                self.eng = eng
                self.raws, self.done = {}, set()

            def dma(self, i):
                if i < h_tiles and i not in self.raws and i not in self.done:
                    t = wrawp.tile([P, H], BF16, tag="wraw", name=f"{self.name}r{i}")
                    self.qs[i % len(self.qs)].dma_start(t[:], self.src_fn(i))
                    self.raws[i] = t

            def cast(self, i):
                if i < h_tiles and i not in self.done:
                    self.dma(i)
                    if self.eng is None:
                        nc.vector.tensor_scalar_mul(
                            self.dst[:, i, :], self.raws.pop(i)[:], self.scale_fn()
                        )
                    else:
                        nc.scalar.mul(
                            self.dst[:, i, :], self.raws.pop(i)[:], self.scale_fn()
                        )
                    self.done.add(i)

            def flush(self):
                for i in range(h_tiles):
                    self.dma(i)
                for i in range(h_tiles):
                    self.cast(i)

        ones_row = rpool.tile([1, P], F32, tag="ones_row")
        nc.vector.memset(ones_row, 1.0)
        ones_col = rpool.tile([P, 1], BF16, tag="ones_col")
        nc.vector.memset(ones_col, 1.0)
        ident = rpool.tile([P, P], BF16, tag="ident")
        make_identity(nc, ident)

        # small router inputs on the tensor ring (issued at kernel start,
        # dependency-free -> never blocks the PE instruction stream)
        br1t = rpool.tile([P, h_tiles], F32, tag="br1t")
        nc.scalar.dma_start(br1t[:], br1_ap.rearrange("(d p) -> p d", p=P))
        w2r = rpool.tile([P, h_tiles, E], F32, tag="w2r")
        for dj in range(h_tiles):
            nc.scalar.dma_start(w2r[:, dj, :], wr2_ap[bass.ts(dj, P), :])
        br2t = rpool.tile([1, E], F32, tag="br2t")
        nc.scalar.dma_start(br2t[:], br2_ap.rearrange("(a e) -> a e", a=1))
        ber0 = bepool.tile([P, H], F32, tag="ber", name="ber0")
        nc.gpsimd.dma_start(ber0[:], be_ap[0:1, :].to_broadcast([P, H]))
        w2b = rpool.tile([P, h_tiles, E], BF16, tag="w2b")
        nc.scalar.copy(w2b[:], w2r[:])
        nc.scalar.mul(ber0[:], ber0[:], WS)  # in place: 256*be0

        # persistent SBUF residents
        xT = xtpool.tile([P, h_tiles, s], BF16, tag="xT")
        xT8 = xtpool.tile([P, h_tiles, s], F8, tag="xT8")
        acc = [accpool.tile([P, H], F32, tag=f"acc{i}", name=f"acc{i}") for i in range(t_tiles)]

        wq8_0 = wq8pool.tile([P, h_tiles, H], F8, tag="wq8", name="wq8_0")
        wr1b = wqpool.tile([P, h_tiles, H], BF16, tag="wq", name="wr1b")
        wq1 = wqpool.tile([P, h_tiles, H], BF16, tag="wq", name="wq1")

        ld_we0 = Loader(
            "we0", lambda i: we_ap[0, bass.ts(i, P), :], wq8_0,
            lambda: WS, [nc.scalar],
        )
        ld_we1 = Loader(
            "we1", lambda i: we_ap[1, bass.ts(i, P), :], wq1,
            lambda: WS, [nc.scalar],
        )
        class Wr1Direct:
            def __init__(self):
                self.done = set()

            def dma(self, i):
                if i < h_tiles and i not in self.done:
                    q = nc.sync if i % 2 == 0 else nc.gpsimd
                    q.dma_start(wr1b[:, i, :], wr1_ap[bass.ts(i, P), :])
                    self.done.add(i)

            def flush(self):
                for i in range(h_tiles):
                    self.dma(i)

        ld_wr1 = Wr1Direct()

        def emit_e0(k):
            p0 = ps.tile([P, NF], F32, tag="ps")
            p1 = ps.tile([P, NF], F32, tag="ps")
            nc.scalar.copy(p0[:], ber0[:, 0:NF])
            nc.scalar.copy(p1[:], ber0[:, NF:H])
            for jp in range(hp):
                lhs = xT8[:, 2 * jp : 2 * jp + 2, bass.ts(k, P)]
                nc.tensor.matmul(
                    p0[:], lhs, wq8_0[:, 2 * jp : 2 * jp + 2, 0:NF],
                    start=False, stop=(jp == hp - 1), perf_mode=DR,
                    skip_group_check=True,
                )
                nc.tensor.matmul(
                    p1[:], lhs, wq8_0[:, 2 * jp : 2 * jp + 2, NF:H],
                    start=False, stop=(jp == hp - 1), perf_mode=DR,
                    skip_group_check=True,
                )
            nc.vector.tensor_scalar_max(acc[k][:, 0:NF], p0[:], 0.0)
            nc.vector.tensor_scalar_max(acc[k][:, NF:H], p1[:], 0.0)

        xs_ps = ps_x.tile([33, NF], F32, tag="xs")

        # ---- phase A: stream x, transpose, expert 0 at a 6-tile lag ----
        E0_LAG = 99  # diagnostic: e0 fully after phase A
        for ti in range(t_tiles):
            q = nc.sync if ti % 2 == 0 else nc.gpsimd
            xs = xstage.tile([P, H], BF16, tag="xs")
            q.dma_start(xs[:], x_ap[bass.ts(ti, P), :])
            xb = xs
            for g in range(2):
                pt = ps_t.tile([P, 4, P], BF16, tag="pt")
                for k in range(4):
                    hj = 4 * g + k
                    nc.tensor.transpose(pt[:, k, :], xb[:, bass.ts(hj, P)], ident[:])
                nc.vector.tensor_copy(xT[:, 4 * g : 4 * g + 4, bass.ts(ti, P)], pt[:])
                nc.scalar.copy(xT8[:, 4 * g : 4 * g + 4, bass.ts(ti, P)], pt[:])
            nc.tensor.matmul(
                xs_ps[0:1, :], ones_col[:], xb[:, 0:NF],
                start=(ti == 0), stop=(ti == t_tiles - 1),
            )
            nc.tensor.matmul(
                xs_ps[32:33, :], ones_col[:], xb[:, NF:H],
                start=(ti == 0), stop=(ti == t_tiles - 1),
            )
            if ti < 4:
                ld_we0.dma(ti)
                ld_we0.dma(ti + 4)
            if ti >= 2 and ti % 2 == 0:
                ld_we0.cast((ti - 2) // 2)
            if ti >= 4:
                ld_we1.dma(ti - 4)
            if ti >= 8:
                ld_we1.cast(ti - 8)
            if 8 <= ti < 12:
                ld_wr1.dma(ti - 8)
                ld_wr1.dma(ti - 4)  # pairs on alternating rings
            if ti >= E0_LAG:
                emit_e0(ti - E0_LAG)
        ld_we0.flush()
        ld_we1.flush()
        for i in range(h_tiles):
            ld_wr1.dma(i)
        for k in range(max(t_tiles - E0_LAG, 0), t_tiles):
            emit_e0(k)
        if dbg:
            nc.sync.dma_start(dbg_acce0, acc[min(2, t_tiles - 1)][:])
            nc.sync.dma_start(dbg_wq1, wq1[:, 0, :])
            nc.sync.dma_start(dbg_wq0, wq8_0[:, 6, :])

        # ---- router inputs: xsum accumulated in PSUM during phase A ----
        xmrb = rpool.tile([1, H], BF16, tag="xmrb")
        nc.scalar.mul(xmrb[:, 0:NF], xs_ps[0:1, :], 1.0 / s)
        nc.scalar.mul(xmrb[:, NF:H], xs_ps[32:33, :], 1.0 / s)
        xmT_ps = ps_r.tile([P, h_tiles, 2], BF16, tag="psr", name="xmT_ps")
        for j in range(h_tiles):
            nc.tensor.transpose(
                xmT_ps[:, j, 0:1], xmrb[:, bass.ts(j, P)], ident[0:1, 0:1]
            )
        xmeanb = rpool.tile([P, h_tiles], BF16, tag="xmeanb")
        for j in range(h_tiles):
            nc.scalar.copy(xmeanb[:, j : j + 1], xmT_ps[:, j, 0:1])
        ld_wr1.flush()

        # ---- expert 1 prefix: 2 tiles of bf16 matmuls cover router latency
        def e1_mms(ti):
            p0 = ps.tile([P, NF], F32, tag="ps")
            p1 = ps.tile([P, NF], F32, tag="ps")
            for hj in range(h_tiles):
                lhs = xT[:, hj, bass.ts(ti, P)]
                nc.tensor.matmul(
                    p0[:], lhs, wq1[:, hj, 0:NF],
                    start=(hj == 0), stop=(hj == h_tiles - 1),
                )
                nc.tensor.matmul(
                    p1[:], lhs, wq1[:, hj, NF:H],
                    start=(hj == 0), stop=(hj == h_tiles - 1),
                )
            return p0, p1

        PREFIX = min(2, t_tiles)
        prefix_ps = [(ti,) + e1_mms(ti) for ti in range(PREFIX)]

        # ---- router compute (PE bits interleave with e1's matmul stream)
        hvec_ps = ps_r.tile([P, h_tiles], F32, tag="psr", name="hvec_ps")
        for dj in range(h_tiles):
            for hj in range(h_tiles):
                nc.tensor.matmul(
                    hvec_ps[:, dj : dj + 1],
                    wr1b[:, hj, bass.ts(dj, P)],
                    xmeanb[:, hj : hj + 1],
                    start=(hj == 0),
                    stop=(hj == h_tiles - 1),
                )
        hsb = rpool.tile([P, h_tiles], F32, tag="hsb")
        nc.vector.tensor_add(hsb[:], hvec_ps[:], br1t[:])
        nc.vector.tensor_scalar_max(hsb[:], hsb[:], 0.0)
        hsbb = rpool.tile([P, h_tiles], BF16, tag="hsbb")
        nc.scalar.copy(hsbb[:], hsb[:])
        lg_ps = ps_r.tile([1, E], F32, tag="psr", name="lg_ps")
        for dj in range(h_tiles):
            nc.tensor.matmul(
                lg_ps[:], hsbb[:, dj : dj + 1], w2b[:, dj, :],
                start=(dj == 0), stop=(dj == h_tiles - 1),
            )
        logits = rpool.tile([1, E], F32, tag="logits")
        nc.vector.tensor_add(logits[:], lg_ps[:], br2t[:])
        mx = rpool.tile([1, 1], F32, tag="mx")
        nc.vector.reduce_max(mx[:], logits[:], axis=AX.X)
        nmx = rpool.tile([1, 1], F32, tag="nmx")
        nc.vector.tensor_scalar_mul(nmx[:], mx[:], -1.0)
        ex = rpool.tile([1, E], F32, tag="ex")
        nc.scalar.activation(ex[:], logits[:], ACTF.Exp, bias=nmx[:], scale=1.0)
        sm = rpool.tile([1, 1], F32, tag="sm")
        nc.vector.reduce_sum(sm[:], ex[:], axis=AX.X)
        rinv = rpool.tile([1, 1], F32, tag="rinv")
        nc.vector.reciprocal(rinv[:], sm[:])
        rvec = rpool.tile([1, E], F32, tag="rvec")
        nc.vector.tensor_scalar_mul(rvec[:], ex[:], rinv[:])
        r_ps = ps_r.tile([P, E], F32, tag="psr", name="r_ps")
        nc.tensor.matmul(r_ps[:], ones_row[:], rvec[:], start=True, stop=True)
        rsb = rpool.tile([P, E], F32, tag="rsb")
        nc.scalar.copy(rsb[:], r_ps[:])
        rsb256 = rpool.tile([P, E], F32, tag="rsb256")
        nc.scalar.mul(rsb256[:], rsb[:], WS)
        if dbg:
            nc.sync.dma_start(dbg_rsb, rsb[:])
            nc.sync.dma_start(dbg_xm, xmrb[:])
            for _j in range(h_tiles):
                nc.sync.dma_start(dbg_xt8[:, bass.ts(_j, P)], xT8[:, _j, bass.ts(min(2, t_tiles - 1), P)])

        # ---- expert 1 epilogues + remaining tiles ----
        ber1 = bepool.tile([P, H], F32, tag="ber", name="ber1")
        nc.gpsimd.dma_start(ber1[:], be_ap[1:2, :].to_broadcast([P, H]))
        bep1 = bepool.tile([P, H], F32, tag="ber", name="bep1")
        nc.scalar.mul(bep1[:], ber1[:], rsb256[:, 1:2])

        wq8_2 = wq8pool.tile([P, h_tiles, H], F8, tag="wq8", name="wq8_2")
        ld_we2 = Loader(
            "we2", lambda i: we_ap[2, bass.ts(i, P), :], wq8_2,
            lambda: rsb256[:, 2:3], [nc.scalar],
        )
        for i in range(4):
            ld_we2.dma(i)

        def e1_epis(ti, p0, p1):
            # deferred expert-0 routing weight, then e1's 2-op epilogue
            nc.scalar.mul(acc[ti][:], acc[ti][:], rsb[:, 0:1])
            for dc, pp in ((0, p0), (1, p1)):
                mt = mpool.tile([P, NF], F32, tag="mt")
                nc.vector.scalar_tensor_tensor(
                    mt[:], pp[:], rsb[:, 1:2], bep1[:, bass.ts(dc, NF)],
                    op0=ALU.mult, op1=ALU.add,
                )
                asl = acc[ti][:, bass.ts(dc, NF)]
                nc.vector.scalar_tensor_tensor(
                    asl, mt[:], 0.0, asl, op0=ALU.max, op1=ALU.add
                )

        for ti, p0, p1 in prefix_ps:
            e1_epis(ti, p0, p1)
        for ti in range(PREFIX, t_tiles):
            p0, p1 = e1_mms(ti)
            e1_epis(ti, p0, p1)
            if 4 <= ti < 8:
                ld_we2.dma(ti)
            if ti >= 5:
                ld_we2.cast(ti - 5)
        ld_we2.flush()
        if dbg:
            nc.sync.dma_start(dbg_wq2, wq8_2[:, 0, :])
            nc.sync.dma_start(dbg_acc0, acc[min(2, t_tiles - 1)][:])

        # ---- experts 2..7 ----
        wq_cur = wq8_2
        ld_next = None
        for e in range(2, E):
            is_f8 = e < E - 1
            ber = bepool.tile([P, H], F32, tag="ber", name=f"ber{e}")
            nc.gpsimd.dma_start(ber[:], be_ap[e : e + 1, :].to_broadcast([P, H]))
            nc.scalar.mul(ber[:], ber[:], rsb256[:, e : e + 1])  # 256*r_e*be

            ne = e + 1
            if ne < E:
                if ne < E - 1:
                    wq_next = wq8pool.tile(
                        [P, h_tiles, H], F8, tag="wq8", name=f"wq8_{ne}"
                    )
                else:
                    wq_next = wqpool.tile(
                        [P, h_tiles, H], BF16, tag="wq", name=f"wq{ne}"
                    )
                nq = nc.scalar if ne % 2 == 0 else nc.gpsimd
                ld_next = Loader(
                    f"we{ne}", lambda i, _e=ne: we_ap[_e, bass.ts(i, P), :],
                    wq_next, lambda _e=ne: rsb256[:, _e : _e + 1], [nq],
                )

            for ti in range(t_tiles):
                p0 = ps.tile([P, NF], F32, tag="ps")
                p1 = ps.tile([P, NF], F32, tag="ps")
                nc.scalar.copy(p0[:], ber[:, 0:NF])
                nc.scalar.copy(p1[:], ber[:, NF:H])
                if is_f8:
                    for jp in range(hp):
                        lhs = xT8[:, 2 * jp : 2 * jp + 2, bass.ts(ti, P)]
                        nc.tensor.matmul(
                            p0[:], lhs, wq_cur[:, 2 * jp : 2 * jp + 2, 0:NF],
                            start=False, stop=(jp == hp - 1), perf_mode=DR,
                            skip_group_check=True,
                        )
                        nc.tensor.matmul(
                            p1[:], lhs, wq_cur[:, 2 * jp : 2 * jp + 2, NF:H],
                            start=False, stop=(jp == hp - 1), perf_mode=DR,
                            skip_group_check=True,
                        )
                else:
                    for hj in range(h_tiles):
                        lhs = xT[:, hj, bass.ts(ti, P)]
                        nc.tensor.matmul(
                            p0[:], lhs, wq_cur[:, hj, 0:NF],
                            start=False, stop=(hj == h_tiles - 1),
                            skip_group_check=True,
                        )
                        nc.tensor.matmul(
                            p1[:], lhs, wq_cur[:, hj, NF:H],
                            start=False, stop=(hj == h_tiles - 1),
                            skip_group_check=True,
                        )
                for dc, pp in ((0, p0), (1, p1)):
                    asl = acc[ti][:, bass.ts(dc, NF)]
                    nc.vector.scalar_tensor_tensor(
                        asl, pp[:], 0.0, asl, op0=ALU.max, op1=ALU.add
                    )
                if e == E - 1:
                    nc.vector.tensor_scalar_mul(acc[ti][:], acc[ti][:], 1.0 / WS)
                    nc.sync.dma_start(out_ap[bass.ts(ti, P), :], acc[ti][:])
                elif ti % 2 == 0:
                    ld_next.dma(ti // 2)
                    if ti >= 2:
                        ld_next.cast(ti // 2 - 1)
            if ne < E:
                ld_next.flush()
                wq_cur = wq_next

    nc.compile()
    return nc


_nc_cache = {}


def _get_nc(s):
    if s not in _nc_cache:
        _nc_cache[s] = build_nc(s)
    return _nc_cache[s]


def kernel(x, We, be, Wr1, br1, Wr2, br2):
    import ml_dtypes

    x = np.ascontiguousarray(np.asarray(x, dtype=np.float32).astype(ml_dtypes.bfloat16))
    We = np.ascontiguousarray(np.asarray(We, dtype=np.float32).astype(ml_dtypes.bfloat16))
    be = np.ascontiguousarray(np.asarray(be, dtype=np.float32))
    Wr1 = np.ascontiguousarray(np.asarray(Wr1, dtype=np.float32).astype(ml_dtypes.bfloat16))
    br1 = np.ascontiguousarray(np.asarray(br1, dtype=np.float32))
    Wr2 = np.ascontiguousarray(np.asarray(Wr2, dtype=np.float32))
    br2 = np.ascontiguousarray(np.asarray(br2, dtype=np.float32))

    s = x.shape[1]
    nc = _get_nc(s)
    shared = {"We": We, "be": be, "Wr1": Wr1, "br1": br1, "Wr2": Wr2, "br2": br2}
    in_maps = [{"x": x[c], **shared} for c in range(N_CORES)]
    res = run_bass_kernel_spmd(nc, in_maps, list(range(N_CORES)))
    return np.stack([res.results[c]["out"] for c in range(N_CORES)], axis=0)
